# revision 22
# baseline (speedup 1.0000x reference)
"""Chebyshev positional-embedding expansion kernel for Trainium2 (8 cores).

Computes out[b, s, d] = T_d(xhat[b, s]), xhat = 2*input_ids/max_seq_len - 1,
T_d = Chebyshev polynomial of the first kind, matching the jax.lax.scan
reference recurrence T_n = 2*xhat*T_{n-1} - T_{n-2} to ~1e-3 rel error.

Strategy (per core; batch row b == core id, no communication):
  T_d(cos t) = cos(d*t), so with phi = arccos(xhat)/(2pi) in [0, 0.5]:
      T[s, n] = cos(2pi * n * phi_s) = sin(2pi * cfrac(n*phi_s + 0.25))
  where cfrac(x) = x - round(x).  phi via the branch-free half-angle form
      arccos(x) = 2*atan(sqrt((1-|x|)/(1+|x|)))   (+ quadrant fold on sign)
  Per position build 64-entry f16 tables
      c_b = cfrac(b*phi)    (b = n % 32),   g_a = cfrac(32a*phi)  (a = n//32)
  (phases via K=32 PE matmuls against a host-provided block-diagonal ramp;
  cfrac via DVE round-to-nearest i32 cast + one fused scalar_tensor_tensor
  per half). Two 2-byte DMA-xbar transposes (16 hh each) turn the tables
  into per-hh [64, 128] lhsT blocks, so the first half of the output
  starts streaming while the second half's tables are still being built.
  Then per hh pair, K=64 f16 PE matmuls against a constant one-hot
  selector produce
      y[s, n] = c_{n%32} + g_{n//32} in [-1, 1]
  in PSUM; one DVE ADD_RANGE_WRAP custom op computes cfrac(y + 0.25) in
  [-0.5, 0.5]; one ACT Sin (scale=2pi) writes T to SBUF. Per output
  element: 1 f16 PE col-cycle + 1 DVE op + 1 ACT op -- every engine sits
  under the ~47us/core HBM write roofline, so the kernel is
  output-DMA bound.

Constants (one-hot selector, block-diag ramp, transpose identity) are
precomputed on the host and DMA'd in on the (otherwise idle-at-start)
sync queue; the input row rides the scalar HWDGE queue so it lands first.
A dummy Sin right after the Arctan pulls the Sin table-set load off the
first-output critical path.

Layout: s_local = p*32 + hh -> partition p owns 32 contiguous output rows.
"""

import numpy as np
import ml_dtypes
jnp_bf16 = ml_dtypes.bfloat16

import concourse.bacc as bacc
import concourse.mybir as mybir
from concourse import tile
from concourse.bass_utils import run_bass_kernel_spmd

F32 = mybir.dt.float32
F16 = mybir.dt.float16
BF16 = mybir.dt.bfloat16
I32 = mybir.dt.int32
U32 = mybir.dt.uint32
OP = mybir.AluOpType
AF = mybir.ActivationFunctionType

N_CORES = 8
B, S, D = 8, 4096, 1024
MAX_SEQ_LEN = 4096
S_PER = B * S // N_CORES
P = 128
H = S_PER // P  # 32
PI = float(np.pi)

# output chunk sizes in hh units (sum = 32); first/last small so the DMA
# stream starts early and drains fast
CHUNKS = (1, 1, 2, 4, 4, 4, 4, 4, 4, 2, 1, 1)


def host_constants():
    n = np.arange(D)
    sel = np.zeros((P, D), np.float16)
    sel[0:32] = (n[None, :] % 32 == np.arange(32)[:, None])
    sel[32:64] = (n[None, :] // 32 == np.arange(32)[:, None])
    sel[64:128] = sel[0:64]
    # rhs[32g + k, cg*1024 + hh*32 + u] = delta(k, hh) * (u if cg==0 else 32u)
    # for the K=96 (phi_hi, phi_mid, phi_lo) table matmuls; all entries are
    # small integers, exact in bf16.
    rhs = np.zeros((96, 2 * H * 32), np.float32)
    for k in range(32):
        for g in range(3):
            rhs[32 * g + k, k * 32:(k + 1) * 32] = np.arange(32)
            rhs[32 * g + k, 1024 + k * 32:1024 + (k + 1) * 32] = \
                np.arange(32) * 32
    rhs = rhs.astype(jnp_bf16)
    idn = np.eye(P, dtype=np.float32).astype(jnp_bf16)
    return {"sel": sel, "rhs": rhs, "idn": idn}


def _emit_body(nc, tc, sb, out_pool, x2d, out3, sel_d, bd_d, idn_d):
    # ---------------- input (scalar HWDGE; lands first) ---------------
    X = sb.tile([P, H], F32, tag="X")
    nc.scalar.dma_start(X[:], x2d)

    # ---------------- constants via DMA (sync queue, idle early) ------
    SEL = sb.tile([P, D], F16, tag="SEL")
    RHS = sb.tile([96, 2 * H * 32], BF16, tag="RHS")
    IDN = sb.tile([P, P], BF16, tag="IDN")
    nc.sync.dma_start(SEL[:], sel_d)
    nc.sync.dma_start(RHS[:], bd_d)
    nc.sync.dma_start(IDN[:], idn_d)

    # ---------------- phi = arccos(xhat)/(2pi), all-DVE ---------------
    XH = sb.tile([P, H], F32, tag="XH")
    AX = sb.tile([P, H], F32, tag="AX")
    MN = sb.tile([P, H], F32, tag="MN")
    MX = sb.tile([P, H], F32, tag="MX")
    RC = sb.tile([P, H], F32, tag="RC")
    Z2 = sb.tile([P, H], F32, tag="Z2")
    Z = sb.tile([P, H], F32, tag="Z")
    AT = sb.tile([P, H], F32, tag="AT")
    MM = sb.tile([P, H], F32, tag="MM")
    SG = sb.tile([P, H], F32, tag="SG")
    A0 = sb.tile([P, H], F32, tag="A0")
    F1 = sb.tile([P, H], F32, tag="F1")
    SCR = sb.tile([P, 1], F32, tag="SCR")
    PHI = sb.tile([P, 64], F32, tag="PHI")  # cols 0:32 phi, 32:64 psi=32phi

    nc.vector.tensor_scalar(XH[:], X[:], 1.0 / (MAX_SEQ_LEN / 2), -1.0,
                            OP.mult, OP.add)
    # |x|; mn = 1-|x|; mx = 1+|x|
    nc.vector.tensor_scalar(AX[:].bitcast(U32), XH[:].bitcast(U32),
                            0x7FFFFFFF, None, OP.bitwise_and)
    nc.vector.tensor_scalar(MN[:], AX[:], -1.0, 1.0, OP.mult, OP.add)
    nc.vector.tensor_scalar(MX[:], AX[:], 1.0, 1.0, OP.mult, OP.add)
    nc.vector.reciprocal(RC[:], MX[:])
    nc.vector.tensor_tensor(Z2[:], MN[:], RC[:], OP.mult)
    nc.scalar.activation(Z[:], Z2[:], AF.Sqrt)
    nc.scalar.activation(AT[:], Z[:], AF.Arctan)
    # preload the Sin table set now (off the critical path)
    nc.scalar.activation(SCR[:], Z[0:P, 0:1], AF.Sin)
    # phi = (1-m)/2 + at*(2m-1)/pi,  m = (x >= 0)
    nc.vector.tensor_scalar(MM[:], XH[:], 0.0, None, OP.is_ge)
    nc.vector.tensor_scalar(SG[:], MM[:], 2.0 / PI, -1.0 / PI, OP.mult,
                            OP.add)
    nc.vector.tensor_scalar(A0[:], MM[:], -0.5, 0.5, OP.mult, OP.add)
    nc.vector.tensor_tensor(F1[:], AT[:], SG[:], OP.mult)
    nc.vector.tensor_tensor(PHI[:, 0:H], F1[:], A0[:], OP.add)
    # split phi into three exact bf16 words: phi = ph + pm + pl (+eps)
    PH3 = sb.tile([P, 96], BF16, tag="PH3")
    R1 = sb.tile([P, H], F32, tag="R1")
    R2 = sb.tile([P, H], F32, tag="R2")
    nc.vector.tensor_copy(PH3[:, 0:H], PHI[:, 0:H])
    nc.vector.tensor_tensor(R1[:], PHI[:, 0:H], PH3[:, 0:H], OP.subtract)
    nc.vector.tensor_copy(PH3[:, H:2 * H], R1[:])
    nc.vector.tensor_tensor(R2[:], R1[:], PH3[:, H:2 * H], OP.subtract)
    nc.vector.tensor_copy(PH3[:, 2 * H:3 * H], R2[:])

    # ---------------- tables (f16), built in hh-halves ----------------
    TBL = sb.tile([P, H, 64], F16, tag="TBL")
    TBLT = sb.tile([P, 16, P], F16, tag="TBLT")

    # ONE PSUM pool (tag "Y", 4 bufs x 2 banks): the first three rotation
    # slots host the PHT transpose and the table phases, then the big loop
    # continues the rotation -- WAR hazards on bank reuse land 2+ hh deep
    # into the pipeline where they are absorbed, instead of stalling hh0.
    with tc.tile_pool(name="psB", bufs=1, space="PSUM") as psB:
        PHS = psB.tile([P, D], F32, tag="Y", bufs=4)
        PHT_PS = PHS[:].bitcast(BF16)
        nc.tensor.transpose(PHT_PS[0:96, 0:P], PH3[:], IDN[:])
        PHT = sb.tile([96, P], BF16, tag="PHT")
        nc.scalar.activation(PHT[:], PHT_PS[0:96, 0:P], AF.Copy)

        YC = psB.tile([P, H * 32], F32, tag="Y", bufs=4)
        YG = psB.tile([P, H * 32], F32, tag="Y", bufs=4)
        KC = sb.tile([P, H * 32], I32, tag="KC")
        KG = sb.tile([P, H * 32], I32, tag="KG")
        YC3 = YC[:].rearrange("p (h b) -> p h b", h=H)
        YG3 = YG[:].rearrange("p (h a) -> p h a", h=H)
        KC3 = KC[:].rearrange("p (h b) -> p h b", h=H)
        KG3 = KG[:].rearrange("p (h a) -> p h a", h=H)
        TBL2 = TBL[:].rearrange("p h u -> p (h u)")

        for (g0, g1) in ((0, 4), (4, 8), (8, 16), (16, 32)):  # hh groups
            sl = slice(g0 * 32, g1 * 32)
            hs = slice(g0, g1)
            nc.tensor.matmul(YC[:, sl], PHT[0:96, :], RHS[0:96, sl])
            nc.tensor.matmul(YG[:, sl], PHT[0:96, :],
                             RHS[0:96, 1024 + g0 * 32:1024 + g1 * 32])
            nc.vector.tensor_copy(KC[:, sl], YC[:, sl])
            nc.vector.tensor_copy(KG[:, sl], YG[:, sl])
            nc.vector.scalar_tensor_tensor(TBL[:, hs, 0:32], YC3[:, hs, :],
                                           0.0, KC3[:, hs, :], OP.subtract,
                                           OP.subtract)
            nc.vector.scalar_tensor_tensor(TBL[:, hs, 32:64], YG3[:, hs, :],
                                           0.0, KG3[:, hs, :], OP.subtract,
                                           OP.subtract)
            # 2-byte xbar transpose: TBLT[p, j, q] = TBL2[q, j*128+p]
            nc.sync.dma_start_transpose(TBLT[:, g0 // 2:g1 // 2, :],
                                        TBL2[:, g0 * 64:g1 * 64])

        # ---------------- big loop ----------------
        def emit_hh(hh, Y, yoff):
            lhsT = TBLT[(hh % 2) * 64:(hh % 2) * 64 + 64, hh // 2, :]
            rb = (hh % 2) * 64
            nc.tensor.matmul(Y[:, yoff:yoff + 512], lhsT,
                             SEL[rb:rb + 64, 0:512])
            nc.tensor.matmul(Y[:, yoff + 512:yoff + 1024], lhsT,
                             SEL[rb:rb + 64, 512:1024])

        hh = 0
        for nh in CHUNKS:
            OUT = out_pool.tile([P, 4, D], F32, tag="OUT")
            h0 = hh
            for q in range(nh):
                Y = psB.tile([P, D], F32, tag="Y", bufs=4)
                emit_hh(hh, Y, 0)
                # cfrac(y + 0.25): wrap handles the cos->sin quarter turn
                nc.vector.add_range_wrap(Y[:], Y[:], 0.25, 0.5, 1.0)
                nc.scalar.activation(OUT[:, q, :], Y[:], AF.Sin,
                                     bias=0.0, scale=2 * PI)
                hh += 1
            nc.sync.dma_start(out3[:, h0:h0 + nh, :], OUT[:, 0:nh, :])


def build_nc():
    nc = bacc.Bacc("TRN2", target_bir_lowering=False, debug=False,
                   num_devices=N_CORES)
    x = nc.dram_tensor("x", [S_PER], F32, kind="ExternalInput")
    sel_t = nc.dram_tensor("sel", [P, D], F16, kind="ExternalInput")
    bd_t = nc.dram_tensor("rhs", [96, 2 * H * 32], BF16, kind="ExternalInput")
    idn_t = nc.dram_tensor("idn", [P, P], BF16, kind="ExternalInput")
    out = nc.dram_tensor("out", [S_PER, D], F32, kind="ExternalOutput")
    x2d = x.rearrange("(p h) -> p h", p=P)
    out3 = out.rearrange("(p h) d -> p h d", p=P)

    with tile.TileContext(nc) as tc:
        with (
            tc.tile_pool(name="sb", bufs=1) as sb,
            tc.tile_pool(name="outp", bufs=3) as out_pool,
        ):
            _emit_body(nc, tc, sb, out_pool, x2d, out3, sel_t[:, :],
                       bd_t[:, :], idn_t[:, :])

    nc.compile()
    return nc


_CACHED_NC = None


def kernel(input_ids, max_seq_len, d_model):
    """Full-input entry point: shards batch rows across the 8 cores."""
    global _CACHED_NC
    input_ids = np.ascontiguousarray(np.asarray(input_ids, dtype=np.float32))
    assert input_ids.shape == (B, S) and int(max_seq_len) == MAX_SEQ_LEN \
        and int(d_model) == D
    if _CACHED_NC is None:
        _CACHED_NC = build_nc()
    consts = host_constants()
    in_maps = [{"x": input_ids[c], **consts} for c in range(N_CORES)]
    res = run_bass_kernel_spmd(_CACHED_NC, in_maps,
                               core_ids=list(range(N_CORES)))
    return np.stack([res.results[c]["out"] for c in range(N_CORES)], axis=0)


# revision 23
# speedup vs baseline: 1.1278x; 1.1278x over previous
"""Chebyshev positional-embedding expansion kernel for Trainium2 (8 cores).

Computes out[b, s, d] = T_d(xhat[b, s]), xhat = 2*input_ids/max_seq_len - 1,
T_d = Chebyshev polynomial of the first kind, matching the jax.lax.scan
reference recurrence T_n = 2*xhat*T_{n-1} - T_{n-2} to ~1e-3 rel error.

Strategy (per core; batch row b == core id, no communication):
  T_d(cos t) = cos(d*t), so with phi = arccos(xhat)/(2pi) in [0, 0.5]:
      T[s, n] = cos(2pi * n * phi_s) = sin(2pi * cfrac(n*phi_s + 0.25))
  where cfrac(x) = x - round(x).  phi via the branch-free half-angle form
      arccos(x) = 2*atan(sqrt((1-|x|)/(1+|x|)))   (+ quadrant fold on sign)
  on DVE+ACT over the [128, 32] position tile.

  Per position, build 64-entry f16 tables
      c_b = cfrac(b*phi)   (b = n % 32),   g_a = cfrac(32a*phi)  (a = n//32)
  Table phases come from K=96 bf16 PE matmuls: phi is split into three
  exactly-representable bf16 words (hi/mid/lo) so every product with the
  integer ramp entries is exact and accumulates in fp32 PSUM at full
  precision; cfrac = one DVE round-to-nearest i32 cast + one fused
  scalar_tensor_tensor per group. Four 2-byte DMA-xbar transposes
  (4/4/8/16 hh) turn the tables into per-hh [64, 128] lhsT blocks so the
  first output rows start streaming while later tables are still built.

  Main loop, per hh (128 positions x 1024 cols): two K=64 f16 PE matmuls
  against a constant one-hot selector produce
      y[s, n] = c_{n%32} + g_{n//32} in [-1, 1]
  in PSUM; one DVE ADD_RANGE_WRAP custom op computes cfrac(y + 0.25) in
  [-0.5, 0.5]; one ACT Sin (scale=2pi) writes T to SBUF; chunks of 1-4 hh
  stream to HBM on the sync HWDGE queue. Per output element: 1 f16 PE
  col-cycle + 1 DVE op + 1 ACT op -- each engine sits under the
  ~47us/core HBM write roofline, so the kernel is output-DMA bound.

  PSUM is one 8-bank pool (tag Y, 4 bufs x 2 banks): the PHT transpose
  and table phases use the first rotation slots, then the big loop
  continues the rotation, so bank-reuse WARs land deep enough into the
  pipeline to be absorbed.

Constants (one-hot selector, table-ramp rhs, transpose identity) are
precomputed on the host and DMA'd in on the sync queue (idle at start);
the input row rides the scalar HWDGE queue so it lands first. A dummy Sin
right after the Arctan pulls the Sin table-set load off the
first-output critical path.

Layout: s_local = p*32 + hh -> partition p owns 32 contiguous output rows.
"""

import numpy as np
import ml_dtypes
jnp_bf16 = ml_dtypes.bfloat16

import concourse.bacc as bacc
import concourse.mybir as mybir
from concourse import tile
from concourse.bass_utils import run_bass_kernel_spmd

F32 = mybir.dt.float32
F16 = mybir.dt.float16
BF16 = mybir.dt.bfloat16
I32 = mybir.dt.int32
U32 = mybir.dt.uint32
OP = mybir.AluOpType
AF = mybir.ActivationFunctionType

N_CORES = 8
B, S, D = 8, 4096, 1024
MAX_SEQ_LEN = 4096
S_PER = B * S // N_CORES
P = 128
H = S_PER // P  # 32
PI = float(np.pi)

# output chunk sizes in hh units (sum = 32); first/last small so the DMA
# stream starts early and drains fast
CHUNKS = (1, 1, 2, 4, 4, 4, 4, 4, 4, 2, 1, 1)


def host_constants():
    n = np.arange(D)
    sel = np.zeros((P, D), np.float16)
    sel[0:32] = (n[None, :] % 32 == np.arange(32)[:, None])
    sel[32:64] = (n[None, :] // 32 == np.arange(32)[:, None])
    sel[64:128] = sel[0:64]
    # rhs[32g + k, cg*1024 + hh*32 + u] = delta(k, hh) * (u if cg==0 else 32u)
    # for the K=96 (phi_hi, phi_mid, phi_lo) table matmuls; all entries are
    # small integers, exact in bf16.
    rhs = np.zeros((96, 2 * H * 32), np.float32)
    for k in range(32):
        for g in range(3):
            rhs[32 * g + k, k * 32:(k + 1) * 32] = np.arange(32)
            rhs[32 * g + k, 1024 + k * 32:1024 + (k + 1) * 32] = \
                np.arange(32) * 32
    rhs = rhs.astype(jnp_bf16)
    idn = np.eye(P, dtype=np.float32).astype(jnp_bf16)
    return {"sel": sel, "rhs": rhs, "idn": idn}


def _emit_body(nc, tc, sb, out_pool, x2d, out3, sel_d, bd_d, idn_d):
    # ---------------- input (scalar HWDGE; lands first) ---------------
    X = sb.tile([P, H], F32, tag="X")
    nc.scalar.dma_start(X[:], x2d)

    # ---------------- constants via DMA (sync queue, idle early) ------
    SEL = sb.tile([P, D], F16, tag="SEL")
    RHS = sb.tile([96, 2 * H * 32], BF16, tag="RHS")
    IDN = sb.tile([P, P], BF16, tag="IDN")
    nc.sync.dma_start(SEL[:], sel_d)
    nc.sync.dma_start(RHS[:], bd_d)
    nc.sync.dma_start(IDN[:], idn_d)

    # ---------------- phi = arccos(xhat)/(2pi), all-DVE ---------------
    XH = sb.tile([P, H], F32, tag="XH")
    AX = sb.tile([P, H], F32, tag="AX")
    MN = sb.tile([P, H], F32, tag="MN")
    MX = sb.tile([P, H], F32, tag="MX")
    RC = sb.tile([P, H], F32, tag="RC")
    Z2 = sb.tile([P, H], F32, tag="Z2")
    Z = sb.tile([P, H], F32, tag="Z")
    AT = sb.tile([P, H], F32, tag="AT")
    MM = sb.tile([P, H], F32, tag="MM")
    SG = sb.tile([P, H], F32, tag="SG")
    A0 = sb.tile([P, H], F32, tag="A0")
    F1 = sb.tile([P, H], F32, tag="F1")
    SCR = sb.tile([P, 1], F32, tag="SCR")
    PHI = sb.tile([P, 64], F32, tag="PHI")  # cols 0:32 phi, 32:64 psi=32phi

    nc.vector.tensor_scalar(XH[:], X[:], 1.0 / (MAX_SEQ_LEN / 2), -1.0,
                            OP.mult, OP.add)
    # |x|; mn = 1-|x|; mx = 1+|x|
    nc.vector.tensor_scalar(AX[:].bitcast(U32), XH[:].bitcast(U32),
                            0x7FFFFFFF, None, OP.bitwise_and)
    nc.vector.tensor_scalar(MN[:], AX[:], -1.0, 1.0, OP.mult, OP.add)
    nc.vector.tensor_scalar(MX[:], AX[:], 1.0, 1.0, OP.mult, OP.add)
    nc.vector.reciprocal(RC[:], MX[:])
    nc.vector.tensor_tensor(Z2[:], MN[:], RC[:], OP.mult)
    nc.scalar.activation(Z[:], Z2[:], AF.Sqrt)
    nc.scalar.activation(AT[:], Z[:], AF.Arctan)
    # preload the Sin table set now (off the critical path)
    nc.scalar.activation(SCR[:], Z[0:P, 0:1], AF.Sin)
    # phi = (1-m)/2 + at*(2m-1)/pi,  m = (x >= 0)
    nc.vector.tensor_scalar(MM[:], XH[:], 0.0, None, OP.is_ge)
    nc.vector.tensor_scalar(SG[:], MM[:], 2.0 / PI, -1.0 / PI, OP.mult,
                            OP.add)
    nc.vector.tensor_scalar(A0[:], MM[:], -0.5, 0.5, OP.mult, OP.add)
    nc.vector.tensor_tensor(F1[:], AT[:], SG[:], OP.mult)
    nc.vector.tensor_tensor(PHI[:, 0:H], F1[:], A0[:], OP.add)
    # split phi into three exact bf16 words: phi = ph + pm + pl (+eps)
    PH3 = sb.tile([P, 96], BF16, tag="PH3")
    R1 = sb.tile([P, H], F32, tag="R1")
    R2 = sb.tile([P, H], F32, tag="R2")
    nc.vector.tensor_copy(PH3[:, 0:H], PHI[:, 0:H])
    nc.vector.tensor_tensor(R1[:], PHI[:, 0:H], PH3[:, 0:H], OP.subtract)
    nc.vector.tensor_copy(PH3[:, H:2 * H], R1[:])
    nc.vector.tensor_tensor(R2[:], R1[:], PH3[:, H:2 * H], OP.subtract)
    nc.vector.tensor_copy(PH3[:, 2 * H:3 * H], R2[:])

    # ---------------- tables (f16), built in hh groups ----------------
    TBL = sb.tile([P, H, 64], F16, tag="TBL")
    TBLT = sb.tile([P, 16, P], F16, tag="TBLT")

    # ONE PSUM pool (tag "Y", 4 bufs x 2 banks): the first three rotation
    # slots host the PHT transpose and the table phases, then the big loop
    # continues the rotation -- WAR hazards on bank reuse land 2+ hh deep
    # into the pipeline where they are absorbed, instead of stalling hh0.
    with tc.tile_pool(name="psB", bufs=1, space="PSUM") as psB:
        PHS = psB.tile([P, D], F32, tag="Y", bufs=4)
        PHT_PS = PHS[:].bitcast(BF16)
        nc.tensor.transpose(PHT_PS[0:96, 0:P], PH3[:], IDN[:])
        PHT = sb.tile([96, P], BF16, tag="PHT")
        nc.scalar.activation(PHT[:], PHT_PS[0:96, 0:P], AF.Copy)

        YC = psB.tile([P, H * 32], F32, tag="Y", bufs=4)
        YG = psB.tile([P, H * 32], F32, tag="Y", bufs=4)
        KC = sb.tile([P, H * 32], I32, tag="KC")
        KG = sb.tile([P, H * 32], I32, tag="KG")
        YC3 = YC[:].rearrange("p (h b) -> p h b", h=H)
        YG3 = YG[:].rearrange("p (h a) -> p h a", h=H)
        KC3 = KC[:].rearrange("p (h b) -> p h b", h=H)
        KG3 = KG[:].rearrange("p (h a) -> p h a", h=H)
        TBL2 = TBL[:].rearrange("p h u -> p (h u)")

        for (g0, g1) in ((0, 4), (4, 8), (8, 16), (16, 32)):  # hh groups
            sl = slice(g0 * 32, g1 * 32)
            hs = slice(g0, g1)
            nc.tensor.matmul(YC[:, sl], PHT[0:96, :], RHS[0:96, sl])
            nc.tensor.matmul(YG[:, sl], PHT[0:96, :],
                             RHS[0:96, 1024 + g0 * 32:1024 + g1 * 32])
            nc.vector.tensor_copy(KC[:, sl], YC[:, sl])
            nc.vector.tensor_copy(KG[:, sl], YG[:, sl])
            nc.vector.scalar_tensor_tensor(TBL[:, hs, 0:32], YC3[:, hs, :],
                                           0.0, KC3[:, hs, :], OP.subtract,
                                           OP.subtract)
            nc.vector.scalar_tensor_tensor(TBL[:, hs, 32:64], YG3[:, hs, :],
                                           0.0, KG3[:, hs, :], OP.subtract,
                                           OP.subtract)
            # 2-byte xbar transpose: TBLT[p, j, q] = TBL2[q, j*128+p]
            nc.sync.dma_start_transpose(TBLT[:, g0 // 2:g1 // 2, :],
                                        TBL2[:, g0 * 64:g1 * 64])

        # ---------------- big loop ----------------
        def emit_hh(hh, Y, yoff):
            lhsT = TBLT[(hh % 2) * 64:(hh % 2) * 64 + 64, hh // 2, :]
            rb = (hh % 2) * 64
            nc.tensor.matmul(Y[:, yoff:yoff + 512], lhsT,
                             SEL[rb:rb + 64, 0:512])
            nc.tensor.matmul(Y[:, yoff + 512:yoff + 1024], lhsT,
                             SEL[rb:rb + 64, 512:1024])

        hh = 0
        for nh in CHUNKS:
            OUT = out_pool.tile([P, 4, D], F32, tag="OUT")
            h0 = hh
            for q in range(nh):
                Y = psB.tile([P, D], F32, tag="Y", bufs=4)
                emit_hh(hh, Y, 0)
                # cfrac(y + 0.25): wrap handles the cos->sin quarter turn
                nc.vector.add_range_wrap(Y[:], Y[:], 0.25, 0.5, 1.0)
                nc.scalar.activation(OUT[:, q, :], Y[:], AF.Sin,
                                     bias=0.0, scale=2 * PI)
                hh += 1
            nc.sync.dma_start(out3[:, h0:h0 + nh, :], OUT[:, 0:nh, :])


def build_nc():
    nc = bacc.Bacc("TRN2", target_bir_lowering=False, debug=False,
                   num_devices=N_CORES)
    x = nc.dram_tensor("x", [S_PER], F32, kind="ExternalInput")
    sel_t = nc.dram_tensor("sel", [P, D], F16, kind="ExternalInput")
    bd_t = nc.dram_tensor("rhs", [96, 2 * H * 32], BF16, kind="ExternalInput")
    idn_t = nc.dram_tensor("idn", [P, P], BF16, kind="ExternalInput")
    out = nc.dram_tensor("out", [S_PER, D], F32, kind="ExternalOutput")
    x2d = x.rearrange("(p h) -> p h", p=P)
    out3 = out.rearrange("(p h) d -> p h d", p=P)

    with tile.TileContext(nc) as tc:
        with (
            tc.tile_pool(name="sb", bufs=1) as sb,
            tc.tile_pool(name="outp", bufs=3) as out_pool,
        ):
            _emit_body(nc, tc, sb, out_pool, x2d, out3, sel_t[:, :],
                       bd_t[:, :], idn_t[:, :])

    nc.compile()
    return nc


_CACHED_NC = None


def kernel(input_ids, max_seq_len, d_model):
    """Full-input entry point: shards batch rows across the 8 cores."""
    global _CACHED_NC
    input_ids = np.ascontiguousarray(np.asarray(input_ids, dtype=np.float32))
    assert input_ids.shape == (B, S) and int(max_seq_len) == MAX_SEQ_LEN \
        and int(d_model) == D
    if _CACHED_NC is None:
        _CACHED_NC = build_nc()
    consts = host_constants()
    in_maps = [{"x": input_ids[c], **consts} for c in range(N_CORES)]
    res = run_bass_kernel_spmd(_CACHED_NC, in_maps,
                               core_ids=list(range(N_CORES)))
    return np.stack([res.results[c]["out"] for c in range(N_CORES)], axis=0)
